# revision 13
# baseline (speedup 1.0000x reference)
"""Black-Scholes 'all' pricing on 8 Trainium2 NeuronCores (Bass/Tile).

kernel(S0, K, T, vt) -> [N, 4] float32 (call, put, digital_call, digital_put)
N = 8_388_608; options sharded contiguously across 8 cores, each core
processing 1M elements as [128 partitions x 8192] in tiles of F=1024.

v3 design, from measured HW rules (probes + v1/v2 traces):
- DVE: all-f16 TENSOR_TENSOR runs 2x (685ns @1024); f32/mixed TT, STT and
  custom ops are 1x (1215-1224ns); a PSUM *input* costs +1070ns (PSUM
  writes are free); strided writes cost +750ns+. So: f32 compute core,
  nothing read from PSUM, all writes contiguous, and a closed f16 island
  in the tail (tp -> call -> put) where the custom AFFINE_MUL ops do the
  f32->f16 conversion for free (1x either way).
- lnpair lives in PSUM: ACT writes to PSUM are free and GPSIMD (which
  computes b from it) reads PSUM without the DVE read penalty. Frees 16KB
  of SBUF, which is what lets the f32 pipeline fit.
- ACT: ~1148ns/1024-op for f32 SBUF; any f16 operand costs +500-850ns.
  All ACT ops stay f32. Rank-3 pair APs merge ln[Sq|Kr] and erf[d1|d2]
  into single ops (saves an init + semaphores each).
- Outputs: four CONTIGUOUS f16 planes (no strided interleave on-chip,
  output DMA halved to 8 MiB/core); host stacks to [N,4] f32 during the
  unshard. f16 rounding adds ~2e-4 relative error vs the 2e-2 gate.
- d2 is computed as (b - 0.5*vt*T)*isv instead of d1 - sv: this kills
  the sv=exp(u/2) ACT op entirely and makes d1/d2 independent (both read
  only SBUF; they write the PSUM dpair, which is free for DVE writes and
  fine for the ACT erf read).
- GPSIMD takes vtt, pc (f16), b and numer2, freeing four DVE slots.
- ln MUST come from `natural_log` (the combined set's ln is ~16x less
  accurate; its error is amplified by isv=1/sqrt(vt*T) up to 100x and
  lands in the digital outputs). exp: `exp_and_others`, erf:
  `sigmoid_and_others`. ACT work is batched per table set in sub-phases
  over groups of G tiles with explicit same-engine dep edges.
"""
import numpy as np

import concourse.bass as bass
import concourse.tile as tile
from concourse import bacc, mybir
from concourse.bass_utils import run_bass_kernel_spmd
from concourse.dve_ops import AFFINE_MUL_REDUCE
from concourse.tile_rust import add_dep_helper

F32 = mybir.dt.float32
F16 = mybir.dt.float16
AF = mybir.ActivationFunctionType
OP = mybir.AluOpType

R = 0.02
Q = 0.01
INV_SQRT2 = 0.7071067811865476

N = 8_388_608
NCORES = 8
P = 128
FD = N // NCORES // P  # 8192

_KEEP_SETS = ("exp_and_others", "sigmoid_and_others", "natural_log")
_orig_get_tables = None

_NC = None
LAST_EXEC_NS = None
LAST_TRACE_DIR = None
TRACE = False


def _patch_act_tables():
    """Blank the membership of every activation-table set except the three
    we use (list order preserved, so act_func_set_id indices into
    act_info.json stay valid) so the table-load pass resolves ln/exp/erf
    to the sets we want."""
    global _orig_get_tables
    import concourse.hw_specs as hw_specs
    if _orig_get_tables is None:
        _orig_get_tables = hw_specs.get_activation_tables

        def patched(arch):
            tabs = _orig_get_tables(arch)
            return {
                name: (fns if name in _KEEP_SETS else set())
                for name, fns in tabs.items()
            }

        hw_specs.get_activation_tables = patched
        bacc.get_activation_tables = patched


def build_bs(FD=FD, F=1024, G=2, P=P):
    from contextlib import ExitStack
    assert FD % F == 0
    _patch_act_tables()
    ntiles = FD // F
    nc = bacc.Bacc("TRN2", target_bir_lowering=False, debug=False,
                   num_devices=NCORES)
    s_d = nc.dram_tensor("s0", [P, FD], F32, kind="ExternalInput").ap()
    k_d = nc.dram_tensor("k", [P, FD], F32, kind="ExternalInput").ap()
    t_d = nc.dram_tensor("t", [P, FD], F32, kind="ExternalInput").ap()
    v_d = nc.dram_tensor("vt", [P, FD], F32, kind="ExternalInput").ap()
    oc_d = nc.dram_tensor("oc", [P, FD], F16, kind="ExternalOutput").ap()
    op_d = nc.dram_tensor("op", [P, FD], F16, kind="ExternalOutput").ap()
    odc_d = nc.dram_tensor("odc", [P, FD], F32, kind="ExternalOutput").ap()
    odp_d = nc.dram_tensor("odp", [P, FD], F32, kind="ExternalOutput").ap()

    def am(out, in0, in1, s0, s1):
        # out = (in0*s0 + s1) * in1
        nc.vector._custom_dve(AFFINE_MUL_REDUCE, out=out, in0=in0, in1=in1,
                              s0=s0, s1=s1)

    with tile.TileContext(nc) as tc, ExitStack() as ctx:
        inp = ctx.enter_context(tc.tile_pool(name="inp", bufs=2))
        pers = ctx.enter_context(tc.tile_pool(name="pers", bufs=2 * G))
        mida = ctx.enter_context(tc.tile_pool(name="mida", bufs=2))
        midc = ctx.enter_context(tc.tile_pool(name="midc", bufs=2))
        perss = ctx.enter_context(tc.tile_pool(name="perss", bufs=2, space="PSUM"))
        midb = ctx.enter_context(tc.tile_pool(name="midb", bufs=2))
        outp = ctx.enter_context(tc.tile_pool(name="outp", bufs=2))

        ngroups = (ntiles + G - 1) // G

        # ACT-stream phase ordering: chain every ACT op of a sub-phase after
        # all ACT ops of the previous sub-phase, so the scheduler cannot
        # interleave different table sets and thrash ACT_TABLE_LOADs.
        prev_phase = []
        cur_phase = []

        def act(*args, **kwargs):
            bi = nc.scalar.activation(*args, **kwargs)
            for p in prev_phase:
                add_dep_helper(bi.ins, p.ins, sync=False,
                               reason="act table phase ordering")
            cur_phase.append(bi)
            return bi

        def end_phase():
            if cur_phase:
                prev_phase[:] = cur_phase
                cur_phase.clear()

        st = {}  # per-tile tensor handles

        def emit_sp3(tiles):
            # (exp_and_others): isv, sv; DVE d1, d2 (f32 dpair) — emitted
            # inside the next group's SP1 phase to share one exp residency.
            for i in tiles:
                z = st[i]
                isv = midc.tile([P, F], F32, tag="isv", bufs=1)
                act(isv[:], z["u"][:], AF.Exp, scale=-0.5)
                dpair = perss.tile([P, 2, F], F32, tag="dp")
                nc.vector.tensor_mul(dpair[:, 0], z["numer"][:], isv[:])
                nc.vector.tensor_mul(dpair[:, 1], z["numer2"][:], isv[:])
                z["dpair"] = dpair

        def emit_sp4(tiles):
            # (sigmoid_and_others): one erf over [d1|d2] (f32); tail with a
            # closed f16 island; DMA out 4 contiguous f16 planes.
            for i in tiles:
                z = st.pop(i)
                sl = slice(i * F, (i + 1) * F)
                ep = midb.tile([P, 2, F], F32, tag="ep", bufs=1)
                act(ep[:], z["dpair"][:], AF.Erf, scale=INV_SQRT2)
                tp = midb.tile([P, 2, F], F32, tag="tp", bufs=1)
                am(tp[:], ep[:], z["sqkr"][:], 0.5, 0.5)
                oc = outp.tile([P, F], F16, tag="oc")
                nc.vector.tensor_sub(oc[:], tp[:, 0], tp[:, 1])
                op_ = outp.tile([P, F], F16, tag="op")
                nc.vector.tensor_add(op_[:], oc[:], z["pc"][:])
                odc = outp.tile([P, F], F32, tag="odc")
                am(odc[:], ep[:, 1], z["dr"][:], 0.5, 0.5)
                odp = outp.tile([P, F], F32, tag="odp")
                am(odp[:], ep[:, 1], z["dr"][:], -0.5, 0.5)
                nc.sync.dma_start(oc_d[:, sl], oc[:])
                nc.sync.dma_start(op_d[:, sl], op_[:])
                nc.sync.dma_start(odc_d[:, sl], odc[:])
                nc.sync.dma_start(odp_d[:, sl], odp[:])

        prev_tiles = None
        for g in range(ngroups):
            lo, hi = g * G, min((g + 1) * G, ntiles)
            tiles = range(lo, hi)
            # ---- SP1 (exp_and_others): [prev group isv/sv] + dq, dr ----
            if prev_tiles is not None:
                emit_sp3(prev_tiles)
            for i in tiles:
                sl = slice(i * F, (i + 1) * F)
                s = inp.tile([P, F], F32, tag="s")
                nc.sync.dma_start(s[:], s_d[:, sl])
                k = inp.tile([P, F], F32, tag="k")
                nc.sync.dma_start(k[:], k_d[:, sl])
                t = inp.tile([P, F], F32, tag="t")
                nc.sync.dma_start(t[:], t_d[:, sl])
                v = inp.tile([P, F], F32, tag="v")
                nc.sync.dma_start(v[:], v_d[:, sl])

                dq = mida.tile([P, F], F32, tag="dq")
                act(dq[:], t[:], AF.Exp, scale=-Q)
                dr = pers.tile([P, F], F32, tag="dr")
                act(dr[:], t[:], AF.Exp, scale=-R)
                vtt = mida.tile([P, F], F32, tag="vtt")
                nc.gpsimd.tensor_mul(vtt[:], t[:], v[:])
                sqkr = pers.tile([P, 2, F], F32, tag="sqkr")
                nc.vector.tensor_mul(sqkr[:, 0], s[:], dq[:])
                nc.vector.tensor_mul(sqkr[:, 1], k[:], dr[:])
                pc = mida.tile([P, F], F16, tag="pc", bufs=2 * G)
                nc.gpsimd.tensor_sub(pc[:], sqkr[:, 1], sqkr[:, 0])
                st[i] = dict(dr=dr, sqkr=sqkr, vtt=vtt, pc=pc)
            end_phase()
            # ---- erf phase for the previous group ----
            if prev_tiles is not None:
                emit_sp4(prev_tiles)
                end_phase()
            # ---- SP2 (natural_log): ln[Sq|Kr], ln vtt; b, numer ----
            for i in tiles:
                z = st[i]
                lnp = mida.tile([P, 2, F], F32, tag="lnp")
                act(lnp[:], z["sqkr"][:], AF.Ln)
                u = midc.tile([P, F], F32, tag="u")
                act(u[:], z["vtt"][:], AF.Ln)
                b = mida.tile([P, F], F32, tag="b")
                nc.vector.tensor_sub(b[:], lnp[:, 0], lnp[:, 1])
                numer = midc.tile([P, F], F32, tag="numer")
                nc.vector.scalar_tensor_tensor(
                    numer[:], z["vtt"][:], 0.5, b[:], OP.mult, OP.add)
                numer2 = midc.tile([P, F], F32, tag="numer2")
                nc.gpsimd.tensor_sub(numer2[:], numer[:], z["vtt"][:])
                z["u"] = u
                z["numer"] = numer
                z["numer2"] = numer2
            end_phase()
            prev_tiles = tiles
        # drain the last group
        emit_sp3(prev_tiles)
        end_phase()
        emit_sp4(prev_tiles)
        end_phase()
    nc.compile()
    return nc


def _get_nc():
    global _NC
    if _NC is None:
        _NC = build_bs()
    return _NC


def kernel(S0, K, T, vt):
    global LAST_EXEC_NS, LAST_TRACE_DIR
    nc = _get_nc()
    arrs = {
        "s0": np.asarray(S0, dtype=np.float32),
        "k": np.asarray(K, dtype=np.float32),
        "t": np.asarray(T, dtype=np.float32),
        "vt": np.asarray(vt, dtype=np.float32),
    }
    shards = []
    for i in range(NCORES):
        sl = slice(i * P * FD, (i + 1) * P * FD)
        shards.append({
            name: np.ascontiguousarray(a[sl].reshape(P, FD))
            for name, a in arrs.items()
        })
    kwargs = {}
    if TRACE:
        import tempfile
        LAST_TRACE_DIR = tempfile.mkdtemp(prefix="bs_trace_")
        kwargs = dict(trace=True, tmpdir=LAST_TRACE_DIR)
    res = run_bass_kernel_spmd(nc, shards, core_ids=list(range(NCORES)),
                               **kwargs)
    LAST_EXEC_NS = res.exec_time_ns
    out = np.empty((N, 4), dtype=np.float32)
    for i in range(NCORES):
        sl = slice(i * P * FD, (i + 1) * P * FD)
        r = res.results[i]
        out[sl, 0] = r["oc"].reshape(-1).astype(np.float32)
        out[sl, 1] = r["op"].reshape(-1).astype(np.float32)
        out[sl, 2] = r["odc"].reshape(-1)
        out[sl, 3] = r["odp"].reshape(-1)
    return out


# revision 15
# speedup vs baseline: 1.1652x; 1.1652x over previous
"""Black-Scholes 'all' pricing on 8 Trainium2 NeuronCores (Bass/Tile).

kernel(S0, K, T, vt) -> [N, 4] float32 (call, put, digital_call, digital_put)
N = 8_388_608; options sharded contiguously across 8 cores, each core
processing 1M elements as [128 partitions x 8192] in tiles of F=1024.

v3 design, from measured HW rules (probes + v1/v2 traces):
- DVE: all-f16 TENSOR_TENSOR runs 2x (685ns @1024); f32/mixed TT, STT and
  custom ops are 1x (1215-1224ns); a PSUM *input* costs +1070ns (PSUM
  writes are free); strided writes cost +750ns+. So: f32 compute core,
  nothing read from PSUM, all writes contiguous, and a closed f16 island
  in the tail (tp -> call -> put) where the custom AFFINE_MUL ops do the
  f32->f16 conversion for free (1x either way).
- lnpair lives in PSUM: ACT writes to PSUM are free and GPSIMD (which
  computes b from it) reads PSUM without the DVE read penalty. Frees 16KB
  of SBUF, which is what lets the f32 pipeline fit.
- ACT: ~1148ns/1024-op for f32 SBUF; any f16 operand costs +500-850ns.
  All ACT ops stay f32. Rank-3 pair APs merge ln[Sq|Kr] and erf[d1|d2]
  into single ops (saves an init + semaphores each).
- Outputs: four CONTIGUOUS f16 planes (no strided interleave on-chip,
  output DMA halved to 8 MiB/core); host stacks to [N,4] f32 during the
  unshard. f16 rounding adds ~2e-4 relative error vs the 2e-2 gate.
- d2 is computed as (b - 0.5*vt*T)*isv instead of d1 - sv: this kills
  the sv=exp(u/2) ACT op entirely and makes d1/d2 independent (both read
  only SBUF; they write the PSUM dpair, which is free for DVE writes and
  fine for the ACT erf read).
- GPSIMD takes vtt, pc (f16), b and numer2, freeing four DVE slots.
- ln MUST come from `natural_log` (the combined set's ln is ~16x less
  accurate; its error is amplified by isv=1/sqrt(vt*T) up to 100x and
  lands in the digital outputs). exp: `exp_and_others`, erf:
  `sigmoid_and_others`. ACT work is batched per table set in sub-phases
  over groups of G tiles with explicit same-engine dep edges.
"""
import numpy as np

import concourse.bass as bass
import concourse.tile as tile
from concourse import bacc, mybir
from concourse.bass_utils import run_bass_kernel_spmd
from concourse.dve_ops import AFFINE_MUL_REDUCE
from concourse.tile_rust import add_dep_helper

F32 = mybir.dt.float32
F16 = mybir.dt.float16
AF = mybir.ActivationFunctionType
OP = mybir.AluOpType

R = 0.02
Q = 0.01
INV_SQRT2 = 0.7071067811865476

N = 8_388_608
NCORES = 8
P = 128
FD = N // NCORES // P  # 8192

_KEEP_SETS = ("exp_and_others", "sigmoid_and_others", "natural_log")
_orig_get_tables = None

_NC = None
LAST_EXEC_NS = None
LAST_TRACE_DIR = None
TRACE = False


def _patch_act_tables():
    """Blank the membership of every activation-table set except the three
    we use (list order preserved, so act_func_set_id indices into
    act_info.json stay valid) so the table-load pass resolves ln/exp/erf
    to the sets we want."""
    global _orig_get_tables
    import concourse.hw_specs as hw_specs
    if _orig_get_tables is None:
        _orig_get_tables = hw_specs.get_activation_tables

        def patched(arch):
            tabs = _orig_get_tables(arch)
            return {
                name: (fns if name in _KEEP_SETS else set())
                for name, fns in tabs.items()
            }

        hw_specs.get_activation_tables = patched
        bacc.get_activation_tables = patched


def build_bs(FD=FD, F=1024, G=2, P=P):
    from contextlib import ExitStack
    assert FD % F == 0
    _patch_act_tables()
    ntiles = FD // F
    nc = bacc.Bacc("TRN2", target_bir_lowering=False, debug=False,
                   num_devices=NCORES)
    s_d = nc.dram_tensor("s0", [P, FD], F32, kind="ExternalInput").ap()
    k_d = nc.dram_tensor("k", [P, FD], F32, kind="ExternalInput").ap()
    t_d = nc.dram_tensor("t", [P, FD], F32, kind="ExternalInput").ap()
    v_d = nc.dram_tensor("vt", [P, FD], F32, kind="ExternalInput").ap()
    oc_d = nc.dram_tensor("oc", [P, FD], F16, kind="ExternalOutput").ap()
    op_d = nc.dram_tensor("op", [P, FD], F16, kind="ExternalOutput").ap()
    odc_d = nc.dram_tensor("odc", [P, FD], F32, kind="ExternalOutput").ap()
    odp_d = nc.dram_tensor("odp", [P, FD], F32, kind="ExternalOutput").ap()

    def am(out, in0, in1, s0, s1):
        # out = (in0*s0 + s1) * in1
        nc.vector._custom_dve(AFFINE_MUL_REDUCE, out=out, in0=in0, in1=in1,
                              s0=s0, s1=s1)

    with tile.TileContext(nc) as tc, ExitStack() as ctx:
        inp = ctx.enter_context(tc.tile_pool(name="inp", bufs=2))
        pers = ctx.enter_context(tc.tile_pool(name="pers", bufs=2 * G))
        mida = ctx.enter_context(tc.tile_pool(name="mida", bufs=2))
        midc = ctx.enter_context(tc.tile_pool(name="midc", bufs=2))
        perss = ctx.enter_context(tc.tile_pool(name="perss", bufs=2))
        psA = ctx.enter_context(tc.tile_pool(name="psA", bufs=1, space="PSUM"))
        midb = ctx.enter_context(tc.tile_pool(name="midb", bufs=2))
        outp = ctx.enter_context(tc.tile_pool(name="outp", bufs=2))

        ngroups = (ntiles + G - 1) // G

        # ACT-stream phase ordering: chain every ACT op of a sub-phase after
        # all ACT ops of the previous sub-phase, so the scheduler cannot
        # interleave different table sets and thrash ACT_TABLE_LOADs.
        prev_phase = []
        cur_phase = []

        def act(*args, **kwargs):
            bi = nc.scalar.activation(*args, **kwargs)
            for p in prev_phase:
                add_dep_helper(bi.ins, p.ins, sync=False,
                               reason="act table phase ordering")
            cur_phase.append(bi)
            return bi

        def end_phase():
            if cur_phase:
                prev_phase[:] = cur_phase
                cur_phase.clear()

        st = {}  # per-tile tensor handles

        def emit_sp3(tiles):
            # (exp_and_others): isv, sv; DVE d1, d2 (f32 dpair) — emitted
            # inside the next group's SP1 phase to share one exp residency.
            for i in tiles:
                z = st[i]
                isv = psA.tile([P, F], F32, tag="isv")
                act(isv[:], z["u"][:], AF.Exp, scale=-0.5)
                dpair = perss.tile([P, 2, F], F16, tag="dp")
                nc.vector.tensor_mul(dpair[:, 0], z["numer"][:], isv[:])
                nc.vector.tensor_mul(dpair[:, 1], z["numer2"][:], isv[:])
                z["dpair"] = dpair

        def emit_sp4(tiles):
            # (sigmoid_and_others): one erf over [d1|d2] (f32); tail with a
            # closed f16 island; DMA out 4 contiguous f16 planes.
            for i in tiles:
                z = st.pop(i)
                sl = slice(i * F, (i + 1) * F)
                ep = midb.tile([P, 2, F], F32, tag="ep", bufs=1)
                act(ep[:], z["dpair"][:], AF.Erf, scale=INV_SQRT2)
                t1 = psA.tile([P, F], F32, tag="t1")
                am(t1[:], ep[:, 0], z["sqkr"][:, 0], 0.5, 0.5)
                t2 = midb.tile([P, F], F32, tag="t2", bufs=1)
                am(t2[:], ep[:, 1], z["sqkr"][:, 1], 0.5, 0.5)
                oc = outp.tile([P, F], F16, tag="oc")
                nc.vector.tensor_sub(oc[:], t1[:], t2[:])
                op_ = outp.tile([P, F], F16, tag="op")
                nc.vector.tensor_add(op_[:], oc[:], z["pc"][:])
                odc = outp.tile([P, F], F32, tag="odc")
                am(odc[:], ep[:, 1], z["dr"][:], 0.5, 0.5)
                odp = outp.tile([P, F], F32, tag="odp")
                am(odp[:], ep[:, 1], z["dr"][:], -0.5, 0.5)
                nc.sync.dma_start(oc_d[:, sl], oc[:])
                nc.sync.dma_start(op_d[:, sl], op_[:])
                nc.sync.dma_start(odc_d[:, sl], odc[:])
                nc.sync.dma_start(odp_d[:, sl], odp[:])

        prev_tiles = None
        for g in range(ngroups):
            lo, hi = g * G, min((g + 1) * G, ntiles)
            tiles = range(lo, hi)
            # ---- SP1 (exp_and_others): [prev group isv/sv] + dq, dr ----
            if prev_tiles is not None:
                emit_sp3(prev_tiles)
            for i in tiles:
                sl = slice(i * F, (i + 1) * F)
                s = inp.tile([P, F], F32, tag="s")
                nc.sync.dma_start(s[:], s_d[:, sl])
                k = inp.tile([P, F], F32, tag="k")
                nc.sync.dma_start(k[:], k_d[:, sl])
                t = inp.tile([P, F], F32, tag="t")
                nc.sync.dma_start(t[:], t_d[:, sl])
                v = inp.tile([P, F], F32, tag="v")
                nc.sync.dma_start(v[:], v_d[:, sl])

                dq = psA.tile([P, F], F32, tag="dq")
                act(dq[:], t[:], AF.Exp, scale=-Q)
                dr = pers.tile([P, F], F32, tag="dr")
                act(dr[:], t[:], AF.Exp, scale=-R)
                vtt = mida.tile([P, F], F32, tag="vtt")
                nc.gpsimd.tensor_mul(vtt[:], t[:], v[:])
                sqkr = pers.tile([P, 2, F], F32, tag="sqkr")
                nc.vector.tensor_mul(sqkr[:, 0], s[:], dq[:])
                nc.vector.tensor_mul(sqkr[:, 1], k[:], dr[:])
                pc = mida.tile([P, F], F16, tag="pc", bufs=2 * G)
                nc.gpsimd.tensor_sub(pc[:], sqkr[:, 1], sqkr[:, 0])
                st[i] = dict(dr=dr, sqkr=sqkr, vtt=vtt, pc=pc)
            end_phase()
            # ---- erf phase for the previous group ----
            if prev_tiles is not None:
                emit_sp4(prev_tiles)
                end_phase()
            # ---- SP2 (natural_log): ln[Sq|Kr], ln vtt; b, numer ----
            for i in tiles:
                z = st[i]
                lnS = mida.tile([P, F], F32, tag="lnS")
                act(lnS[:], z["sqkr"][:, 0], AF.Ln)
                lnK = psA.tile([P, F], F32, tag="lnK")
                act(lnK[:], z["sqkr"][:, 1], AF.Ln)
                u = midc.tile([P, F], F32, tag="u")
                act(u[:], z["vtt"][:], AF.Ln)
                b = mida.tile([P, F], F32, tag="b")
                nc.vector.tensor_sub(b[:], lnS[:], lnK[:])
                numer = midc.tile([P, F], F32, tag="numer")
                nc.vector.scalar_tensor_tensor(
                    numer[:], z["vtt"][:], 0.5, b[:], OP.mult, OP.add)
                numer2 = midc.tile([P, F], F32, tag="numer2")
                nc.gpsimd.tensor_sub(numer2[:], numer[:], z["vtt"][:])
                z["u"] = u
                z["numer"] = numer
                z["numer2"] = numer2
            end_phase()
            prev_tiles = tiles
        # drain the last group
        emit_sp3(prev_tiles)
        end_phase()
        emit_sp4(prev_tiles)
        end_phase()
    nc.compile()
    return nc


def _get_nc():
    global _NC
    if _NC is None:
        _NC = build_bs()
    return _NC


def kernel(S0, K, T, vt):
    global LAST_EXEC_NS, LAST_TRACE_DIR
    nc = _get_nc()
    arrs = {
        "s0": np.asarray(S0, dtype=np.float32),
        "k": np.asarray(K, dtype=np.float32),
        "t": np.asarray(T, dtype=np.float32),
        "vt": np.asarray(vt, dtype=np.float32),
    }
    shards = []
    for i in range(NCORES):
        sl = slice(i * P * FD, (i + 1) * P * FD)
        shards.append({
            name: np.ascontiguousarray(a[sl].reshape(P, FD))
            for name, a in arrs.items()
        })
    kwargs = {}
    if TRACE:
        import tempfile
        LAST_TRACE_DIR = tempfile.mkdtemp(prefix="bs_trace_")
        kwargs = dict(trace=True, tmpdir=LAST_TRACE_DIR)
    res = run_bass_kernel_spmd(nc, shards, core_ids=list(range(NCORES)),
                               **kwargs)
    LAST_EXEC_NS = res.exec_time_ns
    out = np.empty((N, 4), dtype=np.float32)
    for i in range(NCORES):
        sl = slice(i * P * FD, (i + 1) * P * FD)
        r = res.results[i]
        out[sl, 0] = r["oc"].reshape(-1).astype(np.float32)
        out[sl, 1] = r["op"].reshape(-1).astype(np.float32)
        out[sl, 2] = r["odc"].reshape(-1)
        out[sl, 3] = r["odp"].reshape(-1)
    return out
